# revision 6
# baseline (speedup 1.0000x reference)
"""GroupedQueryAttention (B=2, L=2048, D=2048, NH=16, NKV=8, HD=128, RoPE, causal)
sharded tensor-parallel over heads across 8 Trainium2 NeuronCores.

Per core c:
  - owns kv head c and query heads 2c, 2c+1
  - projections q/k/v from full x (each core reads full x, transposed on host),
    with RoPE fused into the projection drain (vector engine) and the V
    transpose done via XBAR DMA-transpose — no separate serial phases
  - attention per head: S^T = k_j.T @ q chunk (keys x queries), exp on scalar
    engine over paired 1024-wide PSUM tiles with causal trimming, AV with a
    ones-column in V so the softmax denominator falls out of the matmul
  - AllToAll redistributes attention outputs head-sharded -> sequence-sharded
  - Wo projection with the full Wo resident in SBUF (loaded once during the
    projection phase); each core computes its 512-row output shard
Host does a pure concat of the 8 output shards.
"""

import sys

sys.path.insert(0, "/opt/trn_rl_repo")

import numpy as np

import concourse.bass as bass
import concourse.bacc as bacc
import concourse.tile as tile
from concourse import mybir
from concourse.bass_utils import run_bass_kernel_spmd

# problem shape (hardcoded)
B, L, D = 2, 2048, 2048
NH, NKV, HD = 16, 8, 128
THETA = 10000.0
SCALE = HD**-0.5
NCORES = 8
HPC = NH // NCORES  # query heads per core = 2
LB = B * L  # 4096
SHARD = LB // NCORES  # 512 output rows per core
NDT = D // 128  # 16 d-tiles
NLT = LB // 128  # 32 bl-tiles
NPAIR = 4  # projection chunk pairs (each pair = 1024 bl cols)
F32 = mybir.dt.float32
BF16 = mybir.dt.bfloat16

TRACE = False
TRACE_ALL_CORES = False
LAST_EXEC_NS = None
LAST_RESULTS = None

_CACHE = {}


def build_bass():
    nc = bacc.Bacc(num_devices=NCORES)

    # ---------------- I/O ----------------
    xT = nc.declare_dram_parameter("xT", [D, LB], BF16, isOutput=False)
    wqT = nc.declare_dram_parameter("wqT", [D, HPC * HD], BF16, isOutput=False)
    wkT = nc.declare_dram_parameter("wkT", [D, HD], BF16, isOutput=False)
    wvT = nc.declare_dram_parameter("wvT", [D, HD], BF16, isOutput=False)
    woT = nc.declare_dram_parameter("woT", [D, D], BF16, isOutput=False)
    cosT = nc.declare_dram_parameter("cosT", [HD, LB], F32, isOutput=False)
    sinT = nc.declare_dram_parameter("sinT", [HD, LB], F32, isOutput=False)
    cmask = nc.declare_dram_parameter("cmask", [128, 128], BF16, isOutput=False)
    out = nc.declare_dram_parameter("out", [SHARD, D], F32, isOutput=True)

    # collective bounce buffers (block j of a2a_in goes to core j)
    a2a_in = [nc.dram_tensor(f"a2a_in{h}", [NCORES, 128, SHARD], BF16) for h in range(HPC)]
    a2a_out = [nc.dram_tensor(f"a2a_out{h}", [NCORES, 128, SHARD], BF16) for h in range(HPC)]

    with tile.TileContext(nc) as tc:
        with tc.tile_pool(name="persist", bufs=1) as persist:
            cmask_sb = persist.tile([128, 128], BF16, tag="cmask", name="cmask")
            nc.sync.dma_start(out=cmask_sb, in_=cmask[:, :])
            # roped projections, bf16, [hd, bl]
            qTb = [persist.tile([128, LB], BF16, tag=f"qTb{h}", name=f"qTb{h}") for h in range(HPC)]
            kTb = persist.tile([128, LB], BF16, tag="kTb", name="kTb")
            # v in [key, hd+1] layout (ones column for softmax denominator)
            v_sb = persist.tile([128, NLT, 144], BF16, tag="v", name="v")
            nc.vector.memset(v_sb[:, :, HD : HD + 1], 1.0)
            # full Wo resident in SBUF: block g = din rows of global head g
            woT_sb = persist.tile([128, NDT, D], BF16, tag="woT", name="woT")

            # ---------------- projections + rope + v transpose ----------------
            with (
                tc.tile_pool(name="wpool", bufs=1) as wp,
                tc.tile_pool(name="tbl", bufs=1) as tbl,
                tc.tile_pool(name="xpool", bufs=1) as xp,
                tc.tile_pool(name="vtb", bufs=2) as vp,
                tc.tile_pool(name="rope", bufs=1) as rp,
                tc.tile_pool(name="pj_ps", bufs=1, space="PSUM") as pj_ps,
            ):
                wq_sb = wp.tile([128, NDT, HPC * HD], BF16, tag="wq", name="wq")
                nc.sync.dma_start(out=wq_sb, in_=wqT.ap().rearrange("(n p) m -> p n m", p=128))
                wk_sb = wp.tile([128, NDT, HD], BF16, tag="wk", name="wk")
                nc.sync.dma_start(out=wk_sb, in_=wkT.ap().rearrange("(n p) m -> p n m", p=128))
                wv_sb = wp.tile([128, NDT, HD], BF16, tag="wv", name="wv")
                nc.sync.dma_start(out=wv_sb, in_=wvT.ap().rearrange("(n p) m -> p n m", p=128))

                # big loads off the sync queue so x tile loads aren't head-blocked
                cos_sb = tbl.tile([128, LB], F32, tag="cos", name="cos")
                nc.scalar.dma_start(out=cos_sb, in_=cosT[:, :])
                sin_sb = tbl.tile([128, LB], F32, tag="sin", name="sin")
                nc.scalar.dma_start(out=sin_sb, in_=sinT[:, :])
                nc.scalar.dma_start(out=woT_sb, in_=woT.ap().rearrange("(n p) m -> p n m", p=128))

                xT_t = xT.ap().rearrange("(n p) m -> p n m", p=128)
                wsrc = [(wq_sb, 0), (wq_sb, HD), (wk_sb, 0), (wv_sb, 0)]
                for pr in range(NPAIR):
                    c0 = pr * 1024
                    xs = []
                    for dt in range(NDT):
                        t = xp.tile([128, 1024], BF16, tag=f"x{dt}", name=f"x{dt}")
                        nc.sync.dma_start(out=t, in_=xT_t[:, dt, c0 : c0 + 1024])
                        xs.append(t)
                    ps = [
                        pj_ps.tile([128, 1024], F32, tag=f"pj{d}", name=f"pj{d}")
                        for d in range(4)
                    ]
                    # dst-major, dt-minor: each weight tile stays stationary for
                    # 2 matmuls, and each dst's psum drains while later dsts run
                    for d, (wsb, woff) in enumerate(wsrc):
                        for dt in range(NDT):
                            for cc in range(2):
                                nc.tensor.matmul(
                                    ps[d][:, cc * 512 : cc * 512 + 512],
                                    lhsT=wsb[:, dt, woff : woff + HD],
                                    rhs=xs[dt][:, cc * 512 : cc * 512 + 512],
                                    start=(dt == 0),
                                    stop=(dt == NDT - 1),
                                )
                        if d < 3:  # q0, q1, k: rope on the vector engine
                            dstT = qTb[d] if d < HPC else kTb
                            rot = rp.tile([128, 1024], F32, tag="rot", name="rot")
                            nc.vector.tensor_copy(out=rot[0:64, :], in_=ps[d][64:128, :])
                            nc.vector.tensor_copy(out=rot[64:128, :], in_=ps[d][0:64, :])
                            nc.vector.tensor_mul(
                                out=rot, in0=rot, in1=sin_sb[:, c0 : c0 + 1024]
                            )
                            tc2 = rp.tile([128, 1024], F32, tag="tc2", name="tc2")
                            nc.vector.tensor_mul(
                                out=tc2, in0=ps[d], in1=cos_sb[:, c0 : c0 + 1024]
                            )
                            nc.vector.tensor_add(
                                out=dstT[:, c0 : c0 + 1024], in0=tc2, in1=rot
                            )
                        else:  # v: bf16 copy + XBAR transpose into [key, hd]
                            vtb = vp.tile([128, 1024], BF16, tag="vtb", name="vtb")
                            nc.vector.tensor_copy(out=vtb, in_=ps[d])
                            for qq in range(8):
                                nc.scalar.dma_start_transpose(
                                    out=v_sb[:, pr * 8 + qq, 0:HD],
                                    in_=vtb[:, qq * 128 : (qq + 1) * 128],
                                )

            # ---------------- attention ----------------
            NQC = L // 512  # 4 query chunks per (b, h)
            with (
                tc.tile_pool(name="s_ps", bufs=2, space="PSUM") as s_ps,
                tc.tile_pool(name="o_ps", bufs=4, space="PSUM") as o_ps,
                tc.tile_pool(name="p_sb", bufs=3) as p_pool,
                tc.tile_pool(name="sm", bufs=8) as sm_pool,
                tc.tile_pool(name="stage", bufs=3) as st_pool,
            ):
                for h in range(HPC):
                    for b in range(B):
                        bc = b * L
                        for ci in range(NQC):
                            q0 = ci * 512
                            ig = 4 * ci + 3  # last row-block of this chunk
                            oacc = [
                                o_ps.tile([128, 129], F32, tag="o", name="o")
                                for _ in range(4)
                            ]
                            for p in range(2 * ci + 2):
                                sps = s_ps.tile([128, 1024], F32, tag="s", name="s")
                                for half in range(2):
                                    j = 2 * p + half
                                    nc.tensor.matmul(
                                        sps[:, half * 512 : half * 512 + 512],
                                        lhsT=kTb[:, bc + j * 128 : bc + (j + 1) * 128],
                                        rhs=qTb[h][:, bc + q0 : bc + q0 + 512],
                                        start=True,
                                        stop=True,
                                    )
                                psb = p_pool.tile([128, 1024], BF16, tag="p", name="p")
                                m0 = 2 * p - 4 * ci  # mask offset of first half
                                if m0 < 0:  # fully causal pair: one wide exp
                                    nc.scalar.activation(
                                        out=psb,
                                        in_=sps,
                                        func=mybir.ActivationFunctionType.Exp,
                                        scale=SCALE,
                                    )
                                else:  # diagonal pair: trim masked columns
                                    for half in range(2):
                                        m = m0 + half
                                        lo = half * 512 + m * 128
                                        hi = half * 512 + 512
                                        nc.scalar.activation(
                                            out=psb[:, lo:hi],
                                            in_=sps[:, lo:hi],
                                            func=mybir.ActivationFunctionType.Exp,
                                            scale=SCALE,
                                        )
                                        nc.vector.tensor_mul(
                                            out=psb[:, lo : lo + 128],
                                            in0=psb[:, lo : lo + 128],
                                            in1=cmask_sb,
                                        )
                                for half in range(2):
                                    j = 2 * p + half
                                    m = j - 4 * ci
                                    for ir in range(4):
                                        if m <= ir:
                                            nc.tensor.matmul(
                                                oacc[ir],
                                                lhsT=psb[
                                                    :,
                                                    half * 512
                                                    + ir * 128 : half * 512
                                                    + (ir + 1) * 128,
                                                ],
                                                rhs=v_sb[:, b * (L // 128) + j, 0 : HD + 1],
                                                start=(j == 0),
                                                stop=(j == 4 * ci + ir),
                                            )
                            stg = st_pool.tile([128, 512], BF16, tag="stg", name="stg")
                            for ir in range(4):
                                rcp = sm_pool.tile([128, 1], F32, tag="rcp", name="rcp")
                                nc.vector.reciprocal(rcp, oacc[ir][:, HD : HD + 1])
                                osb = sm_pool.tile([128, 128], BF16, tag="osb", name="osb")
                                nc.vector.tensor_scalar_mul(
                                    osb, oacc[ir][:, 0:HD], rcp
                                )
                                nc.sync.dma_start_transpose(
                                    out=stg[:, ir * 128 : (ir + 1) * 128], in_=osb
                                )
                            nc.sync.dma_start(
                                out=a2a_in[h][b * NQC + ci, :, :], in_=stg
                            )
                    nc.gpsimd.collective_compute(
                        "AllToAll",
                        mybir.AluOpType.bypass,
                        replica_groups=[list(range(NCORES))],
                        ins=[a2a_in[h][:]],
                        outs=[a2a_out[h][:]],
                    )

            # ---------------- Wo projection for this core's row shard ------
            with (
                tc.tile_pool(name="wo_lhs", bufs=2) as lp,
                tc.tile_pool(name="wo_acc", bufs=1) as ap_,
                tc.tile_pool(name="wo_sb", bufs=3) as op_,
                tc.tile_pool(name="wo_ps", bufs=2, space="PSUM") as wops,
            ):
                acc = [
                    ap_.tile([128, D], BF16, tag=f"acc{t}", name=f"acc{t}")
                    for t in range(4)
                ]
                for h in range(HPC):
                    lhs = []
                    for i in range(NCORES):
                        t = lp.tile([128, SHARD], BF16, tag=f"lhs{i}", name=f"lhs{i}")
                        nc.sync.dma_start(out=t, in_=a2a_out[h][i, :, :])
                        lhs.append(t)
                    for tt in range(4):
                        ps4 = [
                            wops.tile([128, 2, 512], F32, tag=f"wo{n}", name=f"wo{n}")
                            for n in range(2)
                        ]
                        for i in range(NCORES):
                            for n in range(4):
                                nc.tensor.matmul(
                                    ps4[n // 2][:, n % 2, :],
                                    lhsT=lhs[i][:, tt * 128 : (tt + 1) * 128],
                                    rhs=woT_sb[:, 2 * i + h, n * 512 : n * 512 + 512],
                                    start=(i == 0),
                                    stop=(i == NCORES - 1),
                                )
                        for n in range(4):
                            pn = ps4[n // 2][:, n % 2, :]
                            if h == 0:
                                nc.vector.tensor_copy(
                                    out=acc[tt][:, n * 512 : n * 512 + 512], in_=pn
                                )
                            else:
                                osb = op_.tile([128, 512], F32, tag="wosb", name="wosb")
                                nc.vector.tensor_add(
                                    out=osb,
                                    in0=acc[tt][:, n * 512 : n * 512 + 512],
                                    in1=pn,
                                )
                                nc.sync.dma_start(
                                    out=out[tt * 128 : (tt + 1) * 128, n * 512 : n * 512 + 512],
                                    in_=osb,
                                )
    nc.finalize()
    return nc


def _host_inputs(x, Wq, Wk, Wv, Wo):
    import ml_dtypes

    bf16 = ml_dtypes.bfloat16
    xT = np.ascontiguousarray(x.reshape(LB, D).T).astype(bf16)
    woT = np.ascontiguousarray(Wo.T).astype(bf16)

    inv_freq = 1.0 / THETA ** (np.arange(0, HD, 2, dtype=np.float32) / HD)
    t = np.arange(L, dtype=np.float32)
    freqs = np.outer(t, inv_freq)  # [L, 64]
    cos_h = np.cos(freqs).T.astype(np.float32)  # [64, L]
    sin_h = np.sin(freqs).T.astype(np.float32)
    cosT = np.concatenate([cos_h, cos_h], 0)  # [128, L]
    sinT = np.concatenate([-sin_h, sin_h], 0)
    cosT = np.ascontiguousarray(np.concatenate([cosT] * B, 1))  # [128, LB]
    sinT = np.ascontiguousarray(np.concatenate([sinT] * B, 1))

    u = np.arange(128, dtype=np.float32)[None, :]
    p = np.arange(128, dtype=np.float32)[:, None]
    cmask = (u >= p).astype(bf16)

    in_maps = []
    for c in range(NCORES):
        in_maps.append(
            {
                "xT": xT,
                "wqT": np.ascontiguousarray(Wq[256 * c : 256 * (c + 1), :].T).astype(bf16),
                "wkT": np.ascontiguousarray(Wk[128 * c : 128 * (c + 1), :].T).astype(bf16),
                "wvT": np.ascontiguousarray(Wv[128 * c : 128 * (c + 1), :].T).astype(bf16),
                "woT": woT,
                "cosT": cosT,
                "sinT": sinT,
                "cmask": cmask,
            }
        )
    return in_maps


def kernel(x, Wq, Wk, Wv, Wo):
    global LAST_EXEC_NS, LAST_RESULTS
    if "nc" not in _CACHE:
        _CACHE["nc"] = build_bass()
    nc = _CACHE["nc"]
    in_maps = _host_inputs(x, Wq, Wk, Wv, Wo)
    kw = {}
    if TRACE:
        kw["trace"] = True
        if TRACE_ALL_CORES:
            kw["trace_cores"] = list(range(NCORES))
    res = run_bass_kernel_spmd(nc, in_maps, list(range(NCORES)), **kw)
    LAST_EXEC_NS = res.exec_time_ns
    LAST_RESULTS = res
    shards = [res.results[c]["out"] for c in range(NCORES)]
    return np.concatenate(shards, 0).reshape(B, L, D)


def bench(x, Wq, Wk, Wv, Wo, iters=6):
    """Steady-state device timing: pre-placed sharded inputs, repeated exec."""
    import time
    import jax
    from jax.sharding import Mesh, PartitionSpec, NamedSharding
    from jax.experimental.shard_map import shard_map
    from concourse import bass2jax

    if "nc" not in _CACHE:
        _CACHE["nc"] = build_bass()
    nc = _CACHE["nc"]
    in_maps = _host_inputs(x, Wq, Wk, Wv, Wo)

    partition_name = nc.partition_id_tensor.name if nc.partition_id_tensor else None
    in_names, out_names, out_avals, zero_outs = [], [], [], []
    for alloc in nc.m.functions[0].allocations:
        if not isinstance(alloc, mybir.MemoryLocationSet):
            continue
        name = alloc.memorylocations[0].name
        if alloc.kind == "ExternalInput":
            if name != partition_name:
                in_names.append(name)
        elif alloc.kind == "ExternalOutput":
            out_names.append(name)
            shape = tuple(alloc.tensor_shape)
            dtype = mybir.dt.np(alloc.dtype)
            out_avals.append(jax.core.ShapedArray(shape, dtype))
            zero_outs.append(np.zeros(shape, dtype))
    n_params = len(in_names)
    n_outs = len(out_avals)
    in_names_all = in_names + out_names
    if partition_name is not None:
        in_names_all.append(partition_name)

    def _body(*args):
        operands = list(args)
        if partition_name is not None:
            operands.append(bass2jax.partition_id_tensor())
        outs = bass2jax._bass_exec_p.bind(
            *operands,
            out_avals=tuple(out_avals),
            in_names=tuple(in_names_all),
            out_names=tuple(out_names),
            lowering_input_output_aliases=(),
            sim_require_finite=True,
            sim_require_nnan=True,
            nc=nc,
        )
        return tuple(outs)

    devices = jax.devices()[:NCORES]
    mesh = Mesh(np.asarray(devices), ("core",))
    donate = tuple(range(n_params, n_params + n_outs))
    in_specs = (PartitionSpec("core"),) * (n_params + n_outs)
    out_specs = (PartitionSpec("core"),) * n_outs
    fn = jax.jit(
        shard_map(_body, mesh=mesh, in_specs=in_specs, out_specs=out_specs, check_rep=False),
        donate_argnums=donate, keep_unused=True,
    )
    sh = NamedSharding(mesh, PartitionSpec("core"))
    ins = []
    for i, name in enumerate(in_names):
        cat = np.concatenate([np.asarray(in_maps[c][name]) for c in range(NCORES)], axis=0)
        ins.append(jax.device_put(cat, sh))
    zero_sets = []
    for _ in range(iters + 1):
        zero_sets.append([
            jax.device_put(np.zeros((NCORES * z.shape[0], *z.shape[1:]), z.dtype), sh)
            for z in zero_outs
        ])
    # warmup
    out = fn(*ins, *zero_sets[0])
    jax.block_until_ready(out)
    times = []
    for it in range(iters):
        t0 = time.perf_counter()
        out = fn(*ins, *zero_sets[it + 1])
        jax.block_until_ready(out)
        times.append(time.perf_counter() - t0)
    times_us = [t * 1e6 for t in times]
    print("per-iter us:", [f"{t:.0f}" for t in times_us])
    print(f"min {min(times_us):.0f} us  median {sorted(times_us)[len(times_us)//2]:.0f} us")
    return min(times_us)


# revision 7
# speedup vs baseline: 1.2457x; 1.2457x over previous
"""GroupedQueryAttention (B=2, L=2048, D=2048, NH=16, NKV=8, HD=128, RoPE, causal)
sharded tensor-parallel over heads across 8 Trainium2 NeuronCores.

Per core c:
  - owns kv head c and query heads 2c, 2c+1
  - projections q/k/v from full x (each core reads full x, transposed on host),
    with RoPE fused into the projection drain (vector engine) and the V
    transpose done via XBAR DMA-transpose — no separate serial phases
  - attention per head: S^T = k_j.T @ q chunk (keys x queries), exp on scalar
    engine over paired 1024-wide PSUM tiles with causal trimming, AV with a
    ones-column in V so the softmax denominator falls out of the matmul
  - AllToAll redistributes attention outputs head-sharded -> sequence-sharded
  - Wo projection with the full Wo resident in SBUF (loaded once during the
    projection phase); each core computes its 512-row output shard
Host does a pure concat of the 8 output shards.
"""

import sys

sys.path.insert(0, "/opt/trn_rl_repo")

import numpy as np

import concourse.bass as bass
import concourse.bacc as bacc
import concourse.tile as tile
from concourse import mybir
from concourse.bass_utils import run_bass_kernel_spmd

# problem shape (hardcoded)
B, L, D = 2, 2048, 2048
NH, NKV, HD = 16, 8, 128
THETA = 10000.0
SCALE = HD**-0.5
NCORES = 8
HPC = NH // NCORES  # query heads per core = 2
LB = B * L  # 4096
SHARD = LB // NCORES  # 512 output rows per core
NDT = D // 128  # 16 d-tiles
NLT = LB // 128  # 32 bl-tiles
NPAIR = 4  # projection chunk pairs (each pair = 1024 bl cols)
F32 = mybir.dt.float32
BF16 = mybir.dt.bfloat16

TRACE = False
TRACE_ALL_CORES = False
LAST_EXEC_NS = None
LAST_RESULTS = None

_CACHE = {}


def build_bass():
    nc = bacc.Bacc(num_devices=NCORES)

    # ---------------- I/O ----------------
    xT = nc.declare_dram_parameter("xT", [D, LB], BF16, isOutput=False)
    wqT = nc.declare_dram_parameter("wqT", [D, HPC * HD], BF16, isOutput=False)
    wkT = nc.declare_dram_parameter("wkT", [D, HD], BF16, isOutput=False)
    wvT = nc.declare_dram_parameter("wvT", [D, HD], BF16, isOutput=False)
    woT = nc.declare_dram_parameter("woT", [D, D], BF16, isOutput=False)
    cosT = nc.declare_dram_parameter("cosT", [HD, LB], F32, isOutput=False)
    sinT = nc.declare_dram_parameter("sinT", [HD, LB], F32, isOutput=False)
    cmask = nc.declare_dram_parameter("cmask", [128, 128], BF16, isOutput=False)
    ident = nc.declare_dram_parameter("ident", [128, 128], BF16, isOutput=False)
    out = nc.declare_dram_parameter("out", [SHARD, D], F32, isOutput=True)

    # collective bounce buffers (block j of a2a_in goes to core j)
    a2a_in = [nc.dram_tensor(f"a2a_in{h}", [NCORES, 128, SHARD], BF16) for h in range(HPC)]
    a2a_out = [nc.dram_tensor(f"a2a_out{h}", [NCORES, 128, SHARD], BF16) for h in range(HPC)]

    with tile.TileContext(nc) as tc:
        with tc.tile_pool(name="persist", bufs=1) as persist:
            cmask_sb = persist.tile([128, 128], BF16, tag="cmask", name="cmask")
            nc.sync.dma_start(out=cmask_sb, in_=cmask[:, :])
            identb_sb = persist.tile([128, 128], BF16, tag="identb", name="identb")
            nc.sync.dma_start(out=identb_sb, in_=ident[:, :])
            # roped projections, bf16, [hd, bl]
            qTb = [persist.tile([128, LB], BF16, tag=f"qTb{h}", name=f"qTb{h}") for h in range(HPC)]
            kTb = persist.tile([128, LB], BF16, tag="kTb", name="kTb")
            # v in [key, hd+1] layout (ones column for softmax denominator)
            v_sb = persist.tile([128, NLT, 144], BF16, tag="v", name="v")
            nc.vector.memset(v_sb[:, :, HD : HD + 1], 1.0)
            # full Wo resident in SBUF: block g = din rows of global head g
            woT_sb = persist.tile([128, NDT, D], BF16, tag="woT", name="woT")

            # ---------------- projections + rope + v transpose ----------------
            with (
                tc.tile_pool(name="wpool", bufs=1) as wp,
                tc.tile_pool(name="tbl", bufs=1) as tbl,
                tc.tile_pool(name="xpool", bufs=1) as xp,
                tc.tile_pool(name="vtb", bufs=2) as vp,
                tc.tile_pool(name="rope", bufs=1) as rp,
                tc.tile_pool(name="pj_ps", bufs=1, space="PSUM") as pj_ps,
            ):
                wq_sb = wp.tile([128, NDT, HPC * HD], BF16, tag="wq", name="wq")
                nc.sync.dma_start(out=wq_sb, in_=wqT.ap().rearrange("(n p) m -> p n m", p=128))
                wk_sb = wp.tile([128, NDT, HD], BF16, tag="wk", name="wk")
                nc.sync.dma_start(out=wk_sb, in_=wkT.ap().rearrange("(n p) m -> p n m", p=128))
                wv_sb = wp.tile([128, NDT, HD], BF16, tag="wv", name="wv")
                nc.sync.dma_start(out=wv_sb, in_=wvT.ap().rearrange("(n p) m -> p n m", p=128))

                cos_sb = tbl.tile([128, LB], F32, tag="cos", name="cos")
                sin_sb = tbl.tile([128, LB], F32, tag="sin", name="sin")

                xT_t = xT.ap().rearrange("(n p) m -> p n m", p=128)
                wsrc = [(wq_sb, 0), (wq_sb, HD), (wk_sb, 0), (wv_sb, 0)]
                for pr in range(NPAIR):
                    c0 = pr * 1024
                    xs = []
                    for dt in range(NDT):
                        t = xp.tile([128, 1024], BF16, tag=f"x{dt}", name=f"x{dt}")
                        nc.sync.dma_start(out=t, in_=xT_t[:, dt, c0 : c0 + 1024])
                        xs.append(t)
                    if pr == 0:  # tables load behind pair-0 x, ahead of pair 1+
                        nc.sync.dma_start(out=cos_sb, in_=cosT[:, :])
                        nc.sync.dma_start(out=sin_sb, in_=sinT[:, :])
                    ps = [
                        pj_ps.tile([128, 1024], F32, tag=f"pj{d}", name=f"pj{d}")
                        for d in range(4)
                    ]
                    # dst-major, dt-minor: each weight tile stays stationary for
                    # 2 matmuls, and each dst's psum drains while later dsts run
                    for d, (wsb, woff) in enumerate(wsrc):
                        for dt in range(NDT):
                            for cc in range(2):
                                nc.tensor.matmul(
                                    ps[d][:, cc * 512 : cc * 512 + 512],
                                    lhsT=wsb[:, dt, woff : woff + HD],
                                    rhs=xs[dt][:, cc * 512 : cc * 512 + 512],
                                    start=(dt == 0),
                                    stop=(dt == NDT - 1),
                                )
                        if d < 3:  # q0, q1, k: rope on the vector engine
                            dstT = qTb[d] if d < HPC else kTb
                            rot = rp.tile([128, 1024], F32, tag="rot", name="rot")
                            nc.vector.tensor_copy(out=rot[0:64, :], in_=ps[d][64:128, :])
                            nc.vector.tensor_copy(out=rot[64:128, :], in_=ps[d][0:64, :])
                            nc.vector.tensor_mul(
                                out=rot, in0=rot, in1=sin_sb[:, c0 : c0 + 1024]
                            )
                            tc2 = rp.tile([128, 1024], F32, tag="tc2", name="tc2")
                            nc.vector.tensor_mul(
                                out=tc2, in0=ps[d], in1=cos_sb[:, c0 : c0 + 1024]
                            )
                            nc.vector.tensor_add(
                                out=dstT[:, c0 : c0 + 1024], in0=tc2, in1=rot
                            )
                        else:  # v: bf16 copy + XBAR transpose into [key, hd]
                            vtb = vp.tile([128, 1024], BF16, tag="vtb", name="vtb")
                            nc.vector.tensor_copy(out=vtb, in_=ps[d])
                            for qq in range(8):
                                nc.scalar.dma_start_transpose(
                                    out=v_sb[:, pr * 8 + qq, 0:HD],
                                    in_=vtb[:, qq * 128 : (qq + 1) * 128],
                                )
                nc.sync.dma_start(
                    out=woT_sb, in_=woT.ap().rearrange("(n p) m -> p n m", p=128)
                )

            # ---------------- attention ----------------
            NQC = L // 512  # 4 query chunks per (b, h)
            with (
                tc.tile_pool(name="s_ps", bufs=2, space="PSUM") as s_ps,
                tc.tile_pool(name="o_ps", bufs=4, space="PSUM") as o_ps,
                tc.tile_pool(name="t_ps", bufs=2, space="PSUM") as t_ps,
                tc.tile_pool(name="p_sb", bufs=3) as p_pool,
                tc.tile_pool(name="sm", bufs=8) as sm_pool,
                tc.tile_pool(name="stage", bufs=3) as st_pool,
            ):
                for h in range(HPC):
                    for b in range(B):
                        bc = b * L
                        for ci in range(NQC):
                            q0 = ci * 512
                            oacc = [
                                o_ps.tile([128, 129], F32, tag="o", name="o")
                                for _ in range(4)
                            ]
                            for j in range(4 * ci + 4):
                                sps = s_ps.tile([128, 512], F32, tag="s", name="s")
                                nc.tensor.matmul(
                                    sps,
                                    lhsT=kTb[:, bc + j * 128 : bc + (j + 1) * 128],
                                    rhs=qTb[h][:, bc + q0 : bc + q0 + 512],
                                    start=True,
                                    stop=True,
                                )
                                psb = p_pool.tile([128, 512], BF16, tag="p", name="p")
                                m = j - 4 * ci
                                lo = max(m, 0) * 128
                                nc.scalar.activation(
                                    out=psb[:, lo:512],
                                    in_=sps[:, lo:512],
                                    func=mybir.ActivationFunctionType.Exp,
                                    scale=SCALE,
                                )
                                if m >= 0:  # diagonal block: mask 128-wide slice
                                    nc.vector.tensor_mul(
                                        out=psb[:, lo : lo + 128],
                                        in0=psb[:, lo : lo + 128],
                                        in1=cmask_sb,
                                    )
                                for ir in range(4):
                                    if m <= ir:
                                        nc.tensor.matmul(
                                            oacc[ir],
                                            lhsT=psb[:, ir * 128 : (ir + 1) * 128],
                                            rhs=v_sb[:, b * (L // 128) + j, 0 : HD + 1],
                                            start=(j == 0),
                                            stop=(j == 4 * ci + ir),
                                        )
                            stg = st_pool.tile([128, 512], BF16, tag="stg", name="stg")
                            for ir in range(4):
                                rcp = sm_pool.tile([128, 1], F32, tag="rcp", name="rcp")
                                nc.vector.reciprocal(rcp, oacc[ir][:, HD : HD + 1])
                                osb = sm_pool.tile([128, 128], BF16, tag="osb", name="osb")
                                nc.vector.tensor_scalar_mul(
                                    osb, oacc[ir][:, 0:HD], rcp
                                )
                                tps = t_ps.tile([128, 128], BF16, tag="t", name="t")
                                nc.tensor.transpose(tps, osb, identb_sb)
                                nc.vector.tensor_copy(
                                    out=stg[:, ir * 128 : (ir + 1) * 128], in_=tps
                                )
                            nc.sync.dma_start(
                                out=a2a_in[h][b * NQC + ci, :, :], in_=stg
                            )
                    nc.gpsimd.collective_compute(
                        "AllToAll",
                        mybir.AluOpType.bypass,
                        replica_groups=[list(range(NCORES))],
                        ins=[a2a_in[h][:]],
                        outs=[a2a_out[h][:]],
                    )

            # ---------------- Wo projection for this core's row shard ------
            with (
                tc.tile_pool(name="wo_lhs", bufs=2) as lp,
                tc.tile_pool(name="wo_acc", bufs=1) as ap_,
                tc.tile_pool(name="wo_sb", bufs=3) as op_,
                tc.tile_pool(name="wo_ps", bufs=2, space="PSUM") as wops,
            ):
                acc = [
                    ap_.tile([128, D], BF16, tag=f"acc{t}", name=f"acc{t}")
                    for t in range(4)
                ]
                for h in range(HPC):
                    lhs = []
                    for i in range(NCORES):
                        t = lp.tile([128, SHARD], BF16, tag=f"lhs{i}", name=f"lhs{i}")
                        nc.sync.dma_start(out=t, in_=a2a_out[h][i, :, :])
                        lhs.append(t)
                    for tt in range(4):
                        ps4 = [
                            wops.tile([128, 2, 512], F32, tag=f"wo{n}", name=f"wo{n}")
                            for n in range(2)
                        ]
                        for i in range(NCORES):
                            for n in range(4):
                                nc.tensor.matmul(
                                    ps4[n // 2][:, n % 2, :],
                                    lhsT=lhs[i][:, tt * 128 : (tt + 1) * 128],
                                    rhs=woT_sb[:, 2 * i + h, n * 512 : n * 512 + 512],
                                    start=(i == 0),
                                    stop=(i == NCORES - 1),
                                )
                        for n in range(4):
                            pn = ps4[n // 2][:, n % 2, :]
                            if h == 0:
                                nc.vector.tensor_copy(
                                    out=acc[tt][:, n * 512 : n * 512 + 512], in_=pn
                                )
                            else:
                                osb = op_.tile([128, 512], F32, tag="wosb", name="wosb")
                                nc.vector.tensor_add(
                                    out=osb,
                                    in0=acc[tt][:, n * 512 : n * 512 + 512],
                                    in1=pn,
                                )
                                nc.sync.dma_start(
                                    out=out[tt * 128 : (tt + 1) * 128, n * 512 : n * 512 + 512],
                                    in_=osb,
                                )
    nc.finalize()
    return nc


def _host_inputs(x, Wq, Wk, Wv, Wo):
    import ml_dtypes

    bf16 = ml_dtypes.bfloat16
    xT = np.ascontiguousarray(x.reshape(LB, D).T).astype(bf16)
    woT = np.ascontiguousarray(Wo.T).astype(bf16)

    inv_freq = 1.0 / THETA ** (np.arange(0, HD, 2, dtype=np.float32) / HD)
    t = np.arange(L, dtype=np.float32)
    freqs = np.outer(t, inv_freq)  # [L, 64]
    cos_h = np.cos(freqs).T.astype(np.float32)  # [64, L]
    sin_h = np.sin(freqs).T.astype(np.float32)
    cosT = np.concatenate([cos_h, cos_h], 0)  # [128, L]
    sinT = np.concatenate([-sin_h, sin_h], 0)
    cosT = np.ascontiguousarray(np.concatenate([cosT] * B, 1))  # [128, LB]
    sinT = np.ascontiguousarray(np.concatenate([sinT] * B, 1))

    u = np.arange(128, dtype=np.float32)[None, :]
    p = np.arange(128, dtype=np.float32)[:, None]
    cmask = (u >= p).astype(bf16)
    ident = np.eye(128, dtype=np.float32).astype(bf16)

    in_maps = []
    for c in range(NCORES):
        in_maps.append(
            {
                "xT": xT,
                "wqT": np.ascontiguousarray(Wq[256 * c : 256 * (c + 1), :].T).astype(bf16),
                "wkT": np.ascontiguousarray(Wk[128 * c : 128 * (c + 1), :].T).astype(bf16),
                "wvT": np.ascontiguousarray(Wv[128 * c : 128 * (c + 1), :].T).astype(bf16),
                "woT": woT,
                "cosT": cosT,
                "sinT": sinT,
                "cmask": cmask,
                "ident": ident,
            }
        )
    return in_maps


def kernel(x, Wq, Wk, Wv, Wo):
    global LAST_EXEC_NS, LAST_RESULTS
    if "nc" not in _CACHE:
        _CACHE["nc"] = build_bass()
    nc = _CACHE["nc"]
    in_maps = _host_inputs(x, Wq, Wk, Wv, Wo)
    kw = {}
    if TRACE:
        kw["trace"] = True
        if TRACE_ALL_CORES:
            kw["trace_cores"] = list(range(NCORES))
    res = run_bass_kernel_spmd(nc, in_maps, list(range(NCORES)), **kw)
    LAST_EXEC_NS = res.exec_time_ns
    LAST_RESULTS = res
    shards = [res.results[c]["out"] for c in range(NCORES)]
    return np.concatenate(shards, 0).reshape(B, L, D)


def bench(x, Wq, Wk, Wv, Wo, iters=6):
    """Steady-state device timing: pre-placed sharded inputs, repeated exec."""
    import time
    import jax
    from jax.sharding import Mesh, PartitionSpec, NamedSharding
    from jax.experimental.shard_map import shard_map
    from concourse import bass2jax

    if "nc" not in _CACHE:
        _CACHE["nc"] = build_bass()
    nc = _CACHE["nc"]
    in_maps = _host_inputs(x, Wq, Wk, Wv, Wo)

    partition_name = nc.partition_id_tensor.name if nc.partition_id_tensor else None
    in_names, out_names, out_avals, zero_outs = [], [], [], []
    for alloc in nc.m.functions[0].allocations:
        if not isinstance(alloc, mybir.MemoryLocationSet):
            continue
        name = alloc.memorylocations[0].name
        if alloc.kind == "ExternalInput":
            if name != partition_name:
                in_names.append(name)
        elif alloc.kind == "ExternalOutput":
            out_names.append(name)
            shape = tuple(alloc.tensor_shape)
            dtype = mybir.dt.np(alloc.dtype)
            out_avals.append(jax.core.ShapedArray(shape, dtype))
            zero_outs.append(np.zeros(shape, dtype))
    n_params = len(in_names)
    n_outs = len(out_avals)
    in_names_all = in_names + out_names
    if partition_name is not None:
        in_names_all.append(partition_name)

    def _body(*args):
        operands = list(args)
        if partition_name is not None:
            operands.append(bass2jax.partition_id_tensor())
        outs = bass2jax._bass_exec_p.bind(
            *operands,
            out_avals=tuple(out_avals),
            in_names=tuple(in_names_all),
            out_names=tuple(out_names),
            lowering_input_output_aliases=(),
            sim_require_finite=True,
            sim_require_nnan=True,
            nc=nc,
        )
        return tuple(outs)

    devices = jax.devices()[:NCORES]
    mesh = Mesh(np.asarray(devices), ("core",))
    donate = tuple(range(n_params, n_params + n_outs))
    in_specs = (PartitionSpec("core"),) * (n_params + n_outs)
    out_specs = (PartitionSpec("core"),) * n_outs
    fn = jax.jit(
        shard_map(_body, mesh=mesh, in_specs=in_specs, out_specs=out_specs, check_rep=False),
        donate_argnums=donate, keep_unused=True,
    )
    sh = NamedSharding(mesh, PartitionSpec("core"))
    ins = []
    for i, name in enumerate(in_names):
        cat = np.concatenate([np.asarray(in_maps[c][name]) for c in range(NCORES)], axis=0)
        ins.append(jax.device_put(cat, sh))
    zero_sets = []
    for _ in range(iters + 1):
        zero_sets.append([
            jax.device_put(np.zeros((NCORES * z.shape[0], *z.shape[1:]), z.dtype), sh)
            for z in zero_outs
        ])
    # warmup
    out = fn(*ins, *zero_sets[0])
    jax.block_until_ready(out)
    times = []
    for it in range(iters):
        t0 = time.perf_counter()
        out = fn(*ins, *zero_sets[it + 1])
        jax.block_until_ready(out)
        times.append(time.perf_counter() - t0)
    times_us = [t * 1e6 for t in times]
    print("per-iter us:", [f"{t:.0f}" for t in times_us])
    print(f"min {min(times_us):.0f} us  median {sorted(times_us)[len(times_us)//2]:.0f} us")
    return min(times_us)
